# revision 36
# baseline (speedup 1.0000x reference)
"""Multi-head attention (B=4, S=1024, D=1024, H=16) on 8 Trainium2 NeuronCores.

Sharding: core c handles batch b = c//2 and head-group g = c%2 (8 of 16 heads).
Each core computes its heads' Q/K/V projections, attention, and a partial
output projection against its 512 rows of W_o.T; the host sums the two
partials per batch and adds b_o.

Device layout tricks:
- Q/K are produced transposed ([dh, s]) straight out of the projection
  matmuls, so attention scores come out as scoresT[sk, sq] with key
  positions on partitions. The padding mask is then a per-partition bias
  on the fused exp activation (exp(0.125*s - 1e6) == 0 in fp32).
- V is produced in [sk, dh] layout with an extra "ones" column per head, so
  one matmul accumulation yields both the attention numerator (partitions
  0..63) and the softmax denominator (partition 64). A single small
  SBUF->SBUF DMA gathers the 16 denominator rows onto partitions 64..71
  (one per head) for the reciprocal-broadcast matmul.
- 1/denominator is broadcast across the 64 rows of each head with a tiny
  selector matmul on the PE, then folded into the merged heads with one
  elementwise multiply before the output projection.
- All matmuls run as float32r (full fp32 operands, ~4x fp32 speed).
"""

import numpy as np

import concourse.bacc as bacc
import concourse.tile as tile
import concourse.mybir as mybir
from concourse.bass_utils import run_bass_kernel_spmd

F32 = mybir.dt.float32
F32R = mybir.dt.float32r
EXP = mybir.ActivationFunctionType.Exp

B, S, D, H = 4, 1024, 1024, 16
DH = D // H            # 64
G = H // 2             # 8 heads per core
GC = G * DH            # 512 output cols per core
NEG = -1000000.0
P = 128
NDC = D // P           # 8 contraction chunks
NTH = 4                # head-pair tiles (GC/128)


def r(ap):
    return ap.bitcast(F32R)


def build(SKT, with_bq, with_bk, with_bv):
    """Build the SPMD program. SKT = number of 128-row key tiles computed."""
    nc = bacc.Bacc(None, target_bir_lowering=False, debug=False)

    xq = nc.dram_tensor("xq", [D, S], F32R, kind="ExternalInput")    # queries[b].T
    xk = nc.dram_tensor("xk", [D, S], F32R, kind="ExternalInput")    # keys[b].T
    xv = nc.dram_tensor("xv", [D, S], F32R, kind="ExternalInput")    # values[b].T
    wq = nc.dram_tensor("wq", [D, GC], F32R, kind="ExternalInput")   # W_q.T slice
    wk = nc.dram_tensor("wk", [D, GC], F32R, kind="ExternalInput")
    wv = nc.dram_tensor("wv", [D, GC], F32R, kind="ExternalInput")
    wo = nc.dram_tensor("wo", [GC, D], F32R, kind="ExternalInput")   # W_o.T rows
    mkb = nc.dram_tensor("mkb", [S], F32, kind="ExternalInput")     # 0 / -1e6
    esel = nc.dram_tensor("esel", [2, P], F32R, kind="ExternalInput")
    bq = nc.dram_tensor("bq", [GC], F32, kind="ExternalInput")
    bk = nc.dram_tensor("bk", [GC], F32, kind="ExternalInput")
    bv = nc.dram_tensor("bv", [GC], F32R, kind="ExternalInput")
    out = nc.dram_tensor("out", [S, D], F32, kind="ExternalOutput")

    SK = SKT * P

    VW = DH + 1              # 65: per-head V slot width (64 V cols + ones col)

    with tile.TileContext(nc) as tc:
        with tc.tile_pool(name="persist", bufs=1) as persist, \
             tc.tile_pool(name="cst", bufs=1) as cst:
            qts = [persist.tile([P, S], F32R, tag=f"qt{i}", name=f"qt{i}")
                   for i in range(NTH)]                           # QT[dh, sq]
            kts = [persist.tile([P, SK], F32R, tag=f"kt{i}", name=f"kt{i}")
                   for i in range(NTH)]                           # KT[dh, sk]
            vp = persist.tile([P, SKT, G, VW], F32R, tag="vp")    # V + ones cols
            mgs = [persist.tile([P, S], F32R, tag=f"mg{i}", name=f"mg{i}")
                   for i in range(NTH)]                           # merged numerators


            mb = cst.tile([P, SKT], F32, tag="mb")
            # pair selector rows live at partitions 64..65 to line up with
            # the gathered denominator/reciprocal rows.
            es = cst.tile([P, P], F32R, tag="es")
            if with_bq:
                bq_sb = cst.tile([P, NTH], F32, tag="bq")
                nc.sync.dma_start(out=bq_sb[:], in_=bq.rearrange("(t p) -> p t", p=P))
            if with_bk:
                bk_sb = cst.tile([P, NTH], F32, tag="bk")
                nc.sync.dma_start(out=bk_sb[:], in_=bk.rearrange("(t p) -> p t", p=P))
            if with_bv:
                bv_sb = cst.tile([1, GC], F32R, tag="bv")
                nc.sync.dma_start(out=bv_sb[:], in_=bv[None, :])
                ones1f = cst.tile([1, P], F32, tag="ones1f")
                nc.vector.memset(ones1f[:], 1.0)
                ones1 = cst.tile([1, P], F32R, tag="ones1")
                nc.vector.tensor_copy(ones1[:], ones1f[:])

            # ACT exp-table preload: dummy activation so the ~2.7us
            # ACT_TABLE_LOAD happens during the startup DMA wait.
            wtb = cst.tile([1, 16], F32, tag="wtb")
            wtb0 = cst.tile([1, 1], F32, tag="wtb0")
            nc.vector.memset(wtb[:], 0.0)
            nc.vector.memset(wtb0[:], 0.0)
            nc.scalar.activation(wtb[:], wtb[:], EXP, bias=wtb0[:], scale=1.0)

            # V slots: ones column at position DH of every head slot.
            # (memset a plain-f32 staging tile, then DVE-copy per head: the
            # copy casts/rounds to f32r, which the BIR verifier requires for
            # anything feeding an fp32r matmul.)
            scrf = cst.tile([P, 640], F32, tag="scrf")
            nc.vector.memset(scrf[:], 0.001)
            scr = cst.tile([P, 640], F32R, tag="scr")
            nc.vector.tensor_copy(scr[:], scrf[:])

            onesw = cst.tile([P, SKT], F32, tag="onesw")
            nc.vector.memset(onesw[:], 1.0)
            for h in range(G):
                nc.vector.tensor_copy(vp[:, :, h, DH], onesw[:])

            # ---- Phase A: projections ----
            # First-chunk operands live in their own small tiles so the very
            # first matmul only waits on two small DMAs (tile-granularity
            # dependency tracking would otherwise stall it on the bulk loads).
            with tc.tile_pool(name="wts", bufs=1) as wts, \
                 tc.tile_pool(name="xs", bufs=2) as xs, \
                 tc.tile_pool(name="psA", bufs=8, space="PSUM") as psA:
                pswu = psA.tile([P, 512], F32, tag="psA", name="pswu")
                for i in range(20):
                    nc.tensor.matmul(pswu[:], r(scr[:, 0:128]), r(scr[:, 128:640]),
                                     start=(i == 0), stop=(i == 19))

                wq0 = wts.tile([P, GC], F32R, tag="wq0")
                nc.sync.dma_start(out=wq0[:], in_=wq[0:P, :])
                wqr = wts.tile([P, NDC - 1, GC], F32R, tag="wqr")
                for c in range(1, NDC):
                    nc.sync.dma_start(out=wqr[:, c - 1, :], in_=wq[c * P:(c + 1) * P, :])
                wk_sb = wts.tile([P, NDC, GC], F32R, tag="wk")
                wv_sb = wts.tile([P, NDC, GC], F32R, tag="wv")

                def wq_at(c):
                    return wq0[:] if c == 0 else wqr[:, c - 1, :]

                # QT[128*t:+128, sq] = sum_c wq[c, t-slice].T @ xq[c, half]
                for half in range(2):
                    qsl = slice(half * 512, (half + 1) * 512)
                    pss = [psA.tile([P, 512], F32, tag="psA", name=f"psA_{half}_{t}")
                           for t in range(NTH)]
                    xt0 = xs.tile([P, 512], F32R, tag="xq0")
                    nc.sync.dma_start(out=xt0[:], in_=xq[0:P, qsl])
                    xtr = xs.tile([P, NDC - 1, 512], F32R, tag="xqr")
                    for c in range(1, NDC):
                        nc.sync.dma_start(out=xtr[:, c - 1, :],
                                          in_=xq[c * P:(c + 1) * P, qsl])
                    for c in range(NDC):
                        rhs = xt0[:] if c == 0 else xtr[:, c - 1, :]
                        for t in range(NTH):
                            nc.tensor.matmul(
                                pss[t][:], r(wq_at(c)[:, t * P:(t + 1) * P]), r(rhs),
                                start=(c == 0), stop=(c == NDC - 1))
                    for t in range(NTH):
                        if with_bq:
                            nc.vector.tensor_scalar_add(
                                qts[t][:, qsl], pss[t][:], bq_sb[:, t:t + 1])
                        else:
                            nc.vector.tensor_copy(qts[t][:, qsl], pss[t][:])

                for c in range(NDC):
                    nc.sync.dma_start(out=wk_sb[:, c, :], in_=wk[c * P:(c + 1) * P, :])
                kgroups = [(s0, min(512, SK - s0)) for s0 in range(0, SK, 512)]
                for gi, (s0, w) in enumerate(kgroups):
                    pss = [psA.tile([P, 512], F32, tag="psA", name=f"psK_{gi}_{t}")
                           for t in range(NTH)]
                    xtr = xs.tile([P, NDC, 512], F32R, tag="xkr")
                    for c in range(NDC):
                        nc.sync.dma_start(out=xtr[:, c, :w],
                                          in_=xk[c * P:(c + 1) * P, s0:s0 + w])
                    for c in range(NDC):
                        for t in range(NTH):
                            nc.tensor.matmul(
                                pss[t][:, :w], r(wk_sb[:, c, t * P:(t + 1) * P]),
                                r(xtr[:, c, :w]),
                                start=(c == 0), stop=(c == NDC - 1))
                    for t in range(NTH):
                        if with_bk:
                            nc.vector.tensor_scalar_add(
                                kts[t][:, s0:s0 + w], pss[t][:, :w], bk_sb[:, t:t + 1])
                        else:
                            nc.vector.tensor_copy(
                                kts[t][:, s0:s0 + w], pss[t][:, :w])

                # V[sk_tile, dh'] = sum_c xv[c, sk_tile].T @ wv[c, :]  (+ b_v)
                for c in range(NDC):
                    nc.sync.dma_start(out=wv_sb[:, c, :], in_=wv[c * P:(c + 1) * P, :])
                psvs = [psA.tile([P, GC], F32, tag="psA", name=f"psv{st}")
                        for st in range(SKT)]
                for c in range(NDC):
                    xt = xs.tile([P, SK], F32R, tag="xv")
                    nc.sync.dma_start(out=xt[:], in_=xv[c * P:(c + 1) * P, 0:SK])
                    for st in range(SKT):
                        nc.tensor.matmul(psvs[st][:], r(xt[:, st * P:(st + 1) * P]),
                                         r(wv_sb[:, c, :]), start=(c == 0),
                                         stop=(c == NDC - 1 and not with_bv))
                for st in range(SKT):
                    if with_bv:
                        nc.tensor.matmul(psvs[st][:], r(ones1[:]), r(bv_sb[:]),
                                         start=False, stop=True)
                    # scatter per-head 64-col slices into the 65-wide slots
                    nc.vector.tensor_copy(
                        vp[:, st, :, 0:DH],
                        psvs[st][:].rearrange("p (g d) -> p g d", g=G))

            nc.sync.dma_start(out=mb[:], in_=mkb[: SK].rearrange("(t p) -> p t", p=P))
            nc.sync.dma_start(out=es[64:66, :], in_=esel[:, :])

            # ---- Phase B: attention per head pair ----
            # (wo prefetch kicks off here so its DMA overlaps attention compute)
            wop_cm = tc.tile_pool(name="wop", bufs=1)
            wop = wop_cm.__enter__()
            wo_sb = wop.tile([P, NTH, D], F32R, tag="wo")
            for c in range(NTH):
                nc.sync.dma_start(out=wo_sb[:, c, :], in_=wo[c * P:(c + 1) * P, :])
            with tc.tile_pool(name="pp", bufs=2 if SKT <= 6 else 1) as pp, \
                 tc.tile_pool(name="dpool", bufs=2) as dpool, \
                 tc.tile_pool(name="psS", bufs=3, space="PSUM") as psS, \
                 tc.tile_pool(name="psV", bufs=1, space="PSUM") as psV, \
                 tc.tile_pool(name="psR", bufs=1, space="PSUM") as psR:
                psbw = psR.tile([P, 512], F32, tag="psR", name="psbw")
                for i in range(8):
                    nc.tensor.matmul(psbw[:], r(scr[:, 0:128]), r(scr[:, 128:640]),
                                     start=(i == 0), stop=(i == 7))
                for th in range(NTH):
                    he, ho = 2 * th, 2 * th + 1
                    # full [sk, sq=1024] p tiles per head; one fused exp per
                    # (head, sk tile) amortizes ACT's ~352-cycle op overhead.
                    pte = pp.tile([P, SKT, S], F32R, tag="pe")
                    pto = pp.tile([P, SKT, S], F32R, tag="po")
                    for st in range(SKT):
                        ksl = slice(st * P, (st + 1) * P)
                        pse = psS.tile([P, S], F32, tag="psS")
                        pso = psS.tile([P, S], F32, tag="psS", name=f"psSo_{th}_{st}")
                        for half in range(2):
                            qsl = slice(half * 512, (half + 1) * 512)
                            nc.tensor.matmul(pse[:, qsl], r(kts[th][0:64, ksl]),
                                             r(qts[th][0:64, qsl]), start=True, stop=True)
                            nc.tensor.matmul(pso[:, qsl], r(kts[th][64:128, ksl]),
                                             r(qts[th][64:128, qsl]), start=True, stop=True)
                        nc.scalar.activation(pte[:, st, :], pse[:], EXP,
                                             bias=mb[:, st:st + 1], scale=0.125)
                        nc.scalar.activation(pto[:, st, :], pso[:], EXP,
                                             bias=mb[:, st:st + 1], scale=0.125)
                    # attnV per half (both heads), then normalize that half
                    # while the other half's attnV still runs. Reciprocal is
                    # lane-parallel, so spread each half's 1024 denominators
                    # over 8 partitions (gather), recip, and scatter back.
                    dst_t = dpool.tile([P, 2, 2, 512], F32, tag="dstp")
                    rsg_t = dpool.tile([P, 2, 128], F32, tag="rsgp")
                    rcp_t = dpool.tile([P, 2, 128], F32R, tag="rcpp")
                    rst_t = dpool.tile([P, 2, 512], F32R, tag="rstp")
                    for half in range(2):
                        qsl = slice(half * 512, (half + 1) * 512)
                        for hi, (h, pt) in enumerate(((he, pte), (ho, pto))):
                            nv = psV.tile([P, 512], F32, tag="psV")
                            for st in range(SKT):
                                nc.tensor.matmul(nv[0:DH + 1, :], r(vp[:, st, h, :]),
                                                 r(pt[:, st, qsl]),
                                                 start=(st == 0), stop=(st == SKT - 1))
                            nc.vector.tensor_copy(
                                mgs[th][64 * hi:64 * hi + 64, qsl], nv[0:64, :])
                            nc.vector.tensor_copy(dst_t[64:65, hi, half, :],
                                                  nv[64:65, :])
                        for hi2 in range(2):
                            nc.sync.dma_start(
                                out=rsg_t[64 + 4 * hi2:68 + 4 * hi2, half, :],
                                in_=dst_t[64:65, hi2, half, :])
                        with nc.allow_low_precision("softmax denom recip at fp32r"):
                            nc.vector.reciprocal(rcp_t[64:72, half, :],
                                                 rsg_t[64:72, half, :])
                        nc.sync.dma_start(out=rst_t[64:66, half, :],
                                          in_=rcp_t[64:72, half, :])
                        pr = psR.tile([P, 512], F32, tag="psR")
                        nc.tensor.matmul(pr[:], r(es[64:66, :]),
                                         r(rst_t[64:66, half, :]), start=True, stop=True)
                        nc.vector.tensor_mul(mgs[th][:, qsl], mgs[th][:, qsl], pr[:])

            # ---- Phase C: normalize + output projection ----
            with tc.tile_pool(name="ot", bufs=3) as ot, \
                 tc.tile_pool(name="psO", bufs=4, space="PSUM") as psO:
                pscw = psO.tile([P, 512], F32, tag="psO", name="pscw")
                for i in range(6):
                    nc.tensor.matmul(pscw[:], r(scr[:, 0:128]), r(scr[:, 128:640]),
                                     start=(i == 0), stop=(i == 5))
                for qt_i in range(8):
                    sqsl = slice(qt_i * P, (qt_i + 1) * P)
                    for oh in range(2):
                        osl = slice(oh * 512, (oh + 1) * 512)
                        pso = psO.tile([P, 512], F32, tag="psO")
                        for c in range(NTH):
                            nc.tensor.matmul(pso[:], r(mgs[c][:, sqsl]),
                                             r(wo_sb[:, c, osl]),
                                             start=(c == 0), stop=(c == NTH - 1))
                        ob = ot.tile([P, 512], F32, tag="ob")
                        if (qt_i + oh) % 2 == 0:
                            nc.vector.tensor_copy(ob[:], pso[:])
                        else:
                            nc.scalar.copy(ob[:], pso[:])
                        nc.sync.dma_start(out=out[sqsl, osl], in_=ob[:])
            wop_cm.__exit__(None, None, None)

    nc.finalize()
    return nc


_CACHE = {}


def kernel(**inputs):
    queries = np.asarray(inputs["queries"], np.float32)
    keys = np.asarray(inputs["keys"], np.float32)
    values = np.asarray(inputs["values"], np.float32)
    valid_lens = np.asarray(inputs["valid_lens"], np.int32)
    W_q = np.asarray(inputs["W_q"], np.float32)
    W_k = np.asarray(inputs["W_k"], np.float32)
    W_v = np.asarray(inputs["W_v"], np.float32)
    W_o = np.asarray(inputs["W_o"], np.float32)
    b_q = np.asarray(inputs["b_q"], np.float32)
    b_k = np.asarray(inputs["b_k"], np.float32)
    b_v = np.asarray(inputs["b_v"], np.float32)
    b_o = np.asarray(inputs["b_o"], np.float32)

    maxv = int(valid_lens.max())
    SKT = max(1, min(8, -(-maxv // P)))
    with_bq, with_bk, with_bv = bool(b_q.any()), bool(b_k.any()), bool(b_v.any())

    key = (SKT, with_bq, with_bk, with_bv)
    if key not in _CACHE:
        _CACHE[key] = build(SKT, with_bq, with_bk, with_bv)
    nc = _CACHE[key]

    esel = np.zeros((2, P), np.float32)
    esel[0, 0:DH] = 1.0
    esel[1, DH:2 * DH] = 1.0

    col = np.arange(S)
    in_maps = []
    for c in range(8):
        b, g = c // 2, c % 2
        gsl = slice(g * GC, (g + 1) * GC)
        mkb = np.where(col < valid_lens[b], 0.0, NEG).astype(np.float32)
        in_maps.append({
            "xq": np.ascontiguousarray(queries[b].T),
            "xk": np.ascontiguousarray(keys[b].T),
            "xv": np.ascontiguousarray(values[b].T),
            "wq": np.ascontiguousarray(W_q.T[:, gsl]),
            "wk": np.ascontiguousarray(W_k.T[:, gsl]),
            "wv": np.ascontiguousarray(W_v.T[:, gsl]),
            "wo": np.ascontiguousarray(W_o.T[gsl, :]),
            "mkb": mkb,
            "esel": esel,
            "bq": np.ascontiguousarray(b_q[gsl]),
            "bk": np.ascontiguousarray(b_k[gsl]),
            "bv": np.ascontiguousarray(b_v[gsl]),
        })

    res = run_bass_kernel_spmd(nc, in_maps, list(range(8)))
    final = np.empty((B, S, D), np.float32)
    for b in range(B):
        final[b] = res.results[2 * b]["out"] + res.results[2 * b + 1]["out"] + b_o
    return final
